# revision 21
# baseline (speedup 1.0000x reference)
"""Causal self-attention (B=4, T=2048, C=1024, H=16) on 8 TRN2 NeuronCores.

Sharding: tensor-parallel over heads. Each core owns 2 heads:
  - c_attn: output columns (q,k,v dims) for its heads  -> [384, 1024] shard
  - attention: embarrassingly parallel over (B, local heads)
  - c_proj: input rows for its heads -> partial [B,T,C] output, summed on host

Device layouts (host pre-transposed so every matmul contraction dim sits on
SBUF partitions):
  xt   [B, C, T]       x transposed; QKV matmul rhs tiles  [128 k, 512 tok]
  wqkv [128, 8, 384]   wqkv[p,k,n] = W_shard.T[k*128+p, n] (lhsT tiles)
  bqkv [128, 3]        per-partition bias, column n_t
  wp   [128, 1024]     wp[p,c] = W_proj[c, core*128+p]     (proj rhs)

Per-core pipeline per batch b:
  QKV^T [128, 3, 2048] = W.T @ x.T (+bias via DVE tensor_scalar_add)
  V2aug [128 tok, j_t, h, 65] = V^T transposed, ones column appended
  per head h, per 512-col i-superblock, per 128-row j tile (causal only):
    S^T = K_j^T.T @ Q^T        [128 j, w i] PSUM   (w shrinks on diagonal)
    P^T = exp(S^T/64 + mask)   ACT -> SBUF, directly the PV rhs
    Y^T[65, 512] += V2aug_j.T @ P^T    (row 64 = softmax denominator)
  batch-end: rcp8 = 1/denoms (one DVE reciprocal), y *= bcast(rcp8)
  proj partial = y_all.T @ Wp^T -> DMA out [B, T, C]
Host: out = sum(partials) + b_proj.
"""

import os
import sys

import numpy as np

os.environ.setdefault("MYCRO_LOCAL_CACHE", "1")
if "/opt/trn_rl_repo" not in sys.path:
    sys.path.insert(0, "/opt/trn_rl_repo")

B, T, C = 4, 2048, 1024
H, D = 16, 64
N_CORES = 8
HPC = H // N_CORES          # heads per core = 2
NL = HPC * D                # local width per q/k/v = 128
KT = C // 128               # 8 contraction tiles for QKV
NT = 3                      # q, k, v
SW = 512                    # i superblock width
NSB = T // SW               # 4 superblocks per batch
NJT = T // 128              # 16 j tiles per batch
NEG = -1.0e30

# matmul input dtype: bf16 (fastest), f32r (tf32-like), f32 (exact, 4x slow)
KDT = os.environ.get("KERNEL_DTYPE", "bf16")

_cache = {}
LAST_RESULT = None


def _np_mdt():
    if KDT == "bf16":
        import ml_dtypes
        return np.dtype(ml_dtypes.bfloat16)
    return np.dtype(np.float32)


def _build():
    import concourse.tile as tile
    from concourse import bacc, mybir

    dt = mybir.dt
    f32 = dt.float32
    mdt = {"bf16": dt.bfloat16, "f32r": dt.float32r, "f32": f32}[KDT]

    nc = bacc.Bacc("TRN2", target_bir_lowering=False, debug=False,
                   num_devices=N_CORES)

    xt = nc.dram_tensor("xt", [B, C, T], mdt, kind="ExternalInput").ap()
    wqkv = nc.dram_tensor("wqkv", [128, KT, NT * 128], mdt,
                          kind="ExternalInput").ap()
    bqkv = nc.dram_tensor("bqkv", [128, NT], f32, kind="ExternalInput").ap()
    wp = nc.dram_tensor("wp", [128, C], mdt, kind="ExternalInput").ap()
    out = nc.dram_tensor("out", [B, T, C], f32, kind="ExternalOutput").ap()

    np_m = _np_mdt() if KDT == "bf16" else np.float32
    ident_np = np.eye(128).astype(np_m)
    # S^T layout: rows x = j (keys), cols y = i (queries); keep j <= i
    trit_np = np.where(np.arange(128)[:, None] <= np.arange(128)[None, :],
                       np.float32(0.0), np.float32(NEG)).astype(np_m)
    ones_np = np.ones((128, NJT, HPC, 1)).astype(np_m)
    ident_dram = nc.inline_tensor(ident_np, name="ident").ap()
    trit_dram = nc.inline_tensor(trit_np, name="tritmask").ap()
    ones_dram = nc.inline_tensor(ones_np, name="onescol").ap()

    Exp = mybir.ActivationFunctionType.Exp

    _alt = [0]

    def copy_alt(dst, src):
        _alt[0] ^= 1
        if _alt[0]:
            nc.scalar.copy(dst, src)
        else:
            nc.vector.tensor_copy(dst, src)

    with tile.TileContext(nc) as tc:
        with (
            tc.tile_pool(name="consts", bufs=1) as consts,
            tc.tile_pool(name="xtp", bufs=1) as xtp,
            tc.tile_pool(name="qkvtp", bufs=2) as qkvtp,
            tc.tile_pool(name="yp", bufs=1) as yp,
            tc.tile_pool(name="v2p", bufs=2) as v2p,
            tc.tile_pool(name="ptp", bufs=8) as ptp,
            tc.tile_pool(name="stage", bufs=4) as stage,
            tc.tile_pool(name="stats", bufs=2) as stats,
            tc.tile_pool(name="rbp", bufs=2) as rbp,
            tc.tile_pool(name="qkv_ps", bufs=2, space="PSUM") as qkv_ps,
            tc.tile_pool(name="s_ps", bufs=3, space="PSUM") as s_ps,
            tc.tile_pool(name="tr_ps", bufs=1, space="PSUM") as tr_ps,
            tc.tile_pool(name="yo_ps", bufs=2, space="PSUM") as yo_ps,
        ):
            # HAM warm-up primer: dense dummy matmuls with no input deps so
            # the PE clock is at 2.4GHz by the time real work arrives.
            prime = consts.tile([128, SW], mdt if KDT != "f32r" else f32)
            nc.gpsimd.memset(prime[:], 0.25)
            for _ in range(0 if KDT == "f32r" else 24):
                pps = s_ps.tile([128, SW], f32, tag="s")
                nc.tensor.matmul(pps[:], lhsT=prime[:, 0:128], rhs=prime[:],
                                 start=True, stop=True)

            wqkv_sb = consts.tile([128, KT, NT * 128], mdt)
            nc.sync.dma_start(wqkv_sb[:], wqkv[:])
            wp_sb = consts.tile([128, C], mdt)
            nc.sync.dma_start(wp_sb[:], wp[:])
            bias_sb = consts.tile([128, NT], f32)
            nc.sync.dma_start(bias_sb[:], bqkv[:])
            ident_sb = consts.tile([128, 128], mdt)
            nc.sync.dma_start(ident_sb[:], ident_dram[:].bitcast(mdt))
            trit_sb = consts.tile([128, 128], mdt)
            nc.sync.dma_start(trit_sb[:], trit_dram[:].bitcast(mdt))
            for b in range(B):
                # ---- load x^T for this batch, [128, k, T] ----
                xt_sb = xtp.tile([128, KT, T], mdt, tag="xt")
                for k in range(KT):
                    nc.sync.dma_start(xt_sb[:, k, :],
                                      xt[b, k * 128:(k + 1) * 128, :])

                # ---- QKV^T = W.T @ x.T + bias ----
                qkvt = qkvtp.tile([128, NT, T], mdt, tag="qkvt")
                for n_t in range(NT):
                    for ts in range(T // SW):
                        ps = qkv_ps.tile([128, SW], f32, tag="qkv")
                        for k in range(KT):
                            nc.tensor.matmul(
                                ps[:],
                                lhsT=wqkv_sb[:, k, n_t * 128:(n_t + 1) * 128],
                                rhs=xt_sb[:, k, ts * SW:(ts + 1) * SW],
                                start=(k == 0), stop=(k == KT - 1),
                            )
                        _alt[0] ^= 1
                        if _alt[0]:
                            nc.scalar.activation(
                                qkvt[:, n_t, ts * SW:(ts + 1) * SW], ps[:],
                                mybir.ActivationFunctionType.Identity,
                                bias=bias_sb[:, n_t:n_t + 1], scale=1.0)
                        else:
                            nc.vector.tensor_scalar_add(
                                qkvt[:, n_t, ts * SW:(ts + 1) * SW], ps[:],
                                bias_sb[:, n_t:n_t + 1])

                # ---- V2aug: V^T transposed + ones column ----
                v2a = v2p.tile([128, NJT, HPC, 65], mdt, tag="v2a")
                nc.sync.dma_start(v2a[:, :, :, 64:65],
                                  ones_dram[:].bitcast(mdt))
                for j_t in range(NJT):
                    trp = tr_ps.tile([128, 128], mdt, tag="tr")
                    nc.tensor.transpose(
                        trp[:], qkvt[:, 2, j_t * 128:(j_t + 1) * 128],
                        ident_sb[:])
                    copy_alt(v2a[:, j_t, 0, 0:64], trp[:, 0:64])
                    copy_alt(v2a[:, j_t, 1, 0:64], trp[:, 64:128])

                # ---- attention: i_sb outer, heads interleaved ----
                y_sb = yp.tile([128, T], mdt, tag="y")
                for i_sb in range(NSB):
                    for h in range(HPC):
                        q_ap = qkvt[h * 64:(h + 1) * 64, 0, :]
                        k_ap = qkvt[h * 64:(h + 1) * 64, 1, :]
                        yt = yo_ps.tile([65, SW], f32, tag="yo")
                        njt = 4 * (i_sb + 1)
                        for j_t in range(njt):
                            jtl = j_t - 4 * i_sb   # >=0 on the diagonal
                            diag = jtl >= 0
                            w = SW - jtl * 128 if diag else SW
                            i_lo = j_t * 128 if diag else i_sb * SW
                            sp = s_ps.tile([128, SW], f32, tag="s")
                            nc.tensor.matmul(
                                sp[:, :w],
                                lhsT=k_ap[:, j_t * 128:(j_t + 1) * 128],
                                rhs=q_ap[:, i_lo:i_lo + w],
                                start=True, stop=not diag,
                            )
                            if diag:
                                # causal mask added in-PSUM: += I.T @ trit
                                nc.tensor.matmul(
                                    sp[:, 0:128], lhsT=ident_sb[:],
                                    rhs=trit_sb[:], start=False, stop=True)
                            pt = ptp.tile([128, SW], mdt, tag="pt")
                            nc.scalar.activation(
                                pt[:, :w], sp[:, :w], Exp, scale=1.0 / D)
                            nc.tensor.matmul(
                                yt[:, SW - w:SW],
                                lhsT=v2a[:, j_t, h, :],
                                rhs=pt[:, :w],
                                start=(j_t == 0), stop=(j_t == njt - 1),
                            )
                        # normalize: y = yt[0:64] * bcast(1/yt[64])
                        # (denom row bounced to SBUF: the approx-recip
                        # custom op misreads PSUM/base-64 inputs on HW)
                        dnr = stats.tile([1, SW], f32, tag="dnr")
                        nc.vector.tensor_copy(dnr[:], yt[64:65, :])
                        rcp = stats.tile([1, SW], f32, tag="rcp")
                        nc.vector.reciprocal_approx_fast(out=rcp[:], in_=dnr[:])
                        rb = rbp.tile([64, SW], f32, tag="rb")
                        nc.gpsimd.partition_broadcast(rb[:], rcp[:])
                        nc.vector.tensor_mul(
                            y_sb[h * 64:(h + 1) * 64,
                                 i_sb * SW:(i_sb + 1) * SW],
                            yt[0:64, :], rb[:])

                # ---- proj partial ----
                for m_t in range(T // 128):
                    for c_h in range(C // SW):
                        op = yo_ps.tile([128, SW], f32, tag="yo")
                        nc.tensor.matmul(
                            op[:],
                            lhsT=y_sb[:, m_t * 128:(m_t + 1) * 128],
                            rhs=wp_sb[:, c_h * SW:(c_h + 1) * SW],
                            start=True, stop=True,
                        )
                        ost = stage.tile([128, SW], f32, tag="ost")
                        nc.vector.tensor_copy(ost[:], op[:])
                        nc.sync.dma_start(
                            out[b, m_t * 128:(m_t + 1) * 128,
                                c_h * SW:(c_h + 1) * SW], ost[:])

    nc.compile()
    return nc


def _get_nc():
    if "nc" not in _cache:
        _cache["nc"] = _build()
    return _cache["nc"]


def kernel(x, W_attn, b_attn, W_proj, b_proj):
    global LAST_RESULT
    from concourse.bass_utils import run_bass_kernel_spmd

    x = np.asarray(x, dtype=np.float32)
    W_attn = np.asarray(W_attn, dtype=np.float32)
    b_attn = np.asarray(b_attn, dtype=np.float32)
    W_proj = np.asarray(W_proj, dtype=np.float32)
    b_proj = np.asarray(b_proj, dtype=np.float32)

    nc = _get_nc()
    np_m = _np_mdt()

    xt = np.ascontiguousarray(x.transpose(0, 2, 1)).astype(np_m)
    in_maps = []
    for c in range(N_CORES):
        sl = slice(c * NL, (c + 1) * NL)
        w_shard = np.concatenate(
            [W_attn[sl], W_attn[C:2 * C][sl], W_attn[2 * C:][sl]], axis=0)
        # wqkv[p, k, n] = w_shard.T[k*128+p, n]
        wqkv = np.ascontiguousarray(
            w_shard.T.reshape(KT, 128, NT * 128).transpose(1, 0, 2)).astype(np_m)
        b_shard = np.concatenate(
            [b_attn[sl], b_attn[C:2 * C][sl], b_attn[2 * C:][sl]])
        bq = np.ascontiguousarray(b_shard.reshape(NT, 128).T)
        wp_c = np.ascontiguousarray(W_proj[:, sl].T).astype(np_m)
        in_maps.append({"xt": xt, "wqkv": wqkv, "bqkv": bq, "wp": wp_c})

    res = run_bass_kernel_spmd(nc, in_maps, core_ids=list(range(N_CORES)))
    LAST_RESULT = res

    acc = res.results[0]["out"].astype(np.float32)
    for c in range(1, N_CORES):
        acc = acc + res.results[c]["out"]
    return acc + b_proj


# revision 22
# speedup vs baseline: 1.0054x; 1.0054x over previous
"""Causal self-attention (B=4, T=2048, C=1024, H=16) on 8 TRN2 NeuronCores.

Sharding: tensor-parallel over heads. Each core owns 2 heads:
  - c_attn: output columns (q,k,v dims) for its heads  -> [384, 1024] shard
  - attention: embarrassingly parallel over (B, local heads)
  - c_proj: input rows for its heads -> partial [B,T,C] output, summed on host

Device layouts (host pre-transposed so every matmul contraction dim sits on
SBUF partitions):
  xt   [B, C, T]       x transposed; QKV matmul rhs tiles  [128 k, 512 tok]
  wqkv [128, 8, 384]   wqkv[p,k,n] = W_shard.T[k*128+p, n] (lhsT tiles)
  bqkv [128, 3]        per-partition bias, column n_t
  wp   [128, 1024]     wp[p,c] = W_proj[c, core*128+p]     (proj rhs)

Per-core pipeline per batch b:
  QKV^T [128, 3, 2048] = W.T @ x.T (+bias via DVE tensor_scalar_add)
  V2aug [128 tok, j_t, h, 65] = V^T transposed, ones column appended
  per head h, per 512-col i-superblock, per 128-row j tile (causal only):
    S^T = K_j^T.T @ Q^T        [128 j, w i] PSUM   (w shrinks on diagonal)
    P^T = exp(S^T/64 + mask)   ACT -> SBUF, directly the PV rhs
    Y^T[65, 512] += V2aug_j.T @ P^T    (row 64 = softmax denominator)
  batch-end: rcp8 = 1/denoms (one DVE reciprocal), y *= bcast(rcp8)
  proj partial = y_all.T @ Wp^T -> DMA out [B, T, C]
Host: out = sum(partials) + b_proj.
"""

import os
import sys

import numpy as np

os.environ.setdefault("MYCRO_LOCAL_CACHE", "1")
if "/opt/trn_rl_repo" not in sys.path:
    sys.path.insert(0, "/opt/trn_rl_repo")

B, T, C = 4, 2048, 1024
H, D = 16, 64
N_CORES = 8
HPC = H // N_CORES          # heads per core = 2
NL = HPC * D                # local width per q/k/v = 128
KT = C // 128               # 8 contraction tiles for QKV
NT = 3                      # q, k, v
SW = 512                    # i superblock width
NSB = T // SW               # 4 superblocks per batch
NJT = T // 128              # 16 j tiles per batch
NEG = -1.0e30

# matmul input dtype: bf16 (fastest), f32r (tf32-like), f32 (exact, 4x slow)
KDT = os.environ.get("KERNEL_DTYPE", "bf16")

_cache = {}
LAST_RESULT = None


def _np_mdt():
    if KDT == "bf16":
        import ml_dtypes
        return np.dtype(ml_dtypes.bfloat16)
    return np.dtype(np.float32)


def _build():
    import concourse.tile as tile
    from concourse import bacc, mybir

    dt = mybir.dt
    f32 = dt.float32
    mdt = {"bf16": dt.bfloat16, "f32r": dt.float32r, "f32": f32}[KDT]

    nc = bacc.Bacc("TRN2", target_bir_lowering=False, debug=False,
                   num_devices=N_CORES)

    xt = nc.dram_tensor("xt", [B, C, T], mdt, kind="ExternalInput").ap()
    wqkv = nc.dram_tensor("wqkv", [128, KT, NT * 128], mdt,
                          kind="ExternalInput").ap()
    bqkv = nc.dram_tensor("bqkv", [128, NT], f32, kind="ExternalInput").ap()
    wp = nc.dram_tensor("wp", [128, C], mdt, kind="ExternalInput").ap()
    out = nc.dram_tensor("out", [B, T, C], f32, kind="ExternalOutput").ap()

    np_m = _np_mdt() if KDT == "bf16" else np.float32
    ident_np = np.eye(128).astype(np_m)
    # S^T layout: rows x = j (keys), cols y = i (queries); keep j <= i
    trit_np = np.where(np.arange(128)[:, None] <= np.arange(128)[None, :],
                       np.float32(0.0), np.float32(NEG)).astype(np_m)
    ones_np = np.ones((128, NJT, HPC, 1)).astype(np_m)
    ident_dram = nc.inline_tensor(ident_np, name="ident").ap()
    trit_dram = nc.inline_tensor(trit_np, name="tritmask").ap()
    ones_dram = nc.inline_tensor(ones_np, name="onescol").ap()

    Exp = mybir.ActivationFunctionType.Exp

    _alt = [0]

    def copy_alt(dst, src):
        _alt[0] ^= 1
        if _alt[0]:
            nc.scalar.copy(dst, src)
        else:
            nc.vector.tensor_copy(dst, src)

    with tile.TileContext(nc) as tc:
        with (
            tc.tile_pool(name="consts", bufs=1) as consts,
            tc.tile_pool(name="xtp", bufs=1) as xtp,
            tc.tile_pool(name="qkvtp", bufs=2) as qkvtp,
            tc.tile_pool(name="yp", bufs=1) as yp,
            tc.tile_pool(name="v2p", bufs=2) as v2p,
            tc.tile_pool(name="ptp", bufs=8) as ptp,
            tc.tile_pool(name="stage", bufs=4) as stage,
            tc.tile_pool(name="stats", bufs=2) as stats,
            tc.tile_pool(name="rbp", bufs=2) as rbp,
            tc.tile_pool(name="qkv_ps", bufs=2, space="PSUM") as qkv_ps,
            tc.tile_pool(name="s_ps", bufs=3, space="PSUM") as s_ps,
            tc.tile_pool(name="tr_ps", bufs=1, space="PSUM") as tr_ps,
            tc.tile_pool(name="yo_ps", bufs=2, space="PSUM") as yo_ps,
        ):
            # HAM warm-up primer: dense dummy matmuls with no input deps so
            # the PE clock is at 2.4GHz by the time real work arrives.
            prime = consts.tile([128, SW], mdt if KDT != "f32r" else f32)
            nc.gpsimd.memset(prime[:], 0.25)
            for _ in range(0 if KDT == "f32r" else 24):
                pps = s_ps.tile([128, SW], f32, tag="s")
                nc.tensor.matmul(pps[:], lhsT=prime[:, 0:128], rhs=prime[:],
                                 start=True, stop=True)

            wqkv_sb = consts.tile([128, KT, NT * 128], mdt)
            nc.sync.dma_start(wqkv_sb[:], wqkv[:])
            wp_sb = consts.tile([128, C], mdt)
            nc.sync.dma_start(wp_sb[:], wp[:])
            bias_sb = consts.tile([128, NT], f32)
            nc.sync.dma_start(bias_sb[:], bqkv[:])
            ident_sb = consts.tile([128, 128], mdt)
            nc.sync.dma_start(ident_sb[:], ident_dram[:].bitcast(mdt))
            trit_sb = consts.tile([128, 128], mdt)
            nc.sync.dma_start(trit_sb[:], trit_dram[:].bitcast(mdt))
            for b in range(B):
                # ---- load x^T for this batch, [128, k, T] ----
                xt_sb = xtp.tile([128, KT, T], mdt, tag="xt")
                for k in range(KT):
                    nc.sync.dma_start(xt_sb[:, k, :],
                                      xt[b, k * 128:(k + 1) * 128, :])

                # ---- QKV^T = W.T @ x.T + bias ----
                qkvt = qkvtp.tile([128, NT, T], mdt, tag="qkvt")
                for n_t in range(NT):
                    for ts in range(T // SW):
                        ps = qkv_ps.tile([128, SW], f32, tag="qkv")
                        for k in range(KT):
                            nc.tensor.matmul(
                                ps[:],
                                lhsT=wqkv_sb[:, k, n_t * 128:(n_t + 1) * 128],
                                rhs=xt_sb[:, k, ts * SW:(ts + 1) * SW],
                                start=(k == 0), stop=(k == KT - 1),
                            )
                        _alt[0] ^= 1
                        if _alt[0]:
                            nc.scalar.activation(
                                qkvt[:, n_t, ts * SW:(ts + 1) * SW], ps[:],
                                mybir.ActivationFunctionType.Identity,
                                bias=bias_sb[:, n_t:n_t + 1], scale=1.0)
                        else:
                            nc.vector.tensor_scalar_add(
                                qkvt[:, n_t, ts * SW:(ts + 1) * SW], ps[:],
                                bias_sb[:, n_t:n_t + 1])

                # ---- V2aug: V^T transposed + ones column ----
                v2a = v2p.tile([128, NJT, HPC, 65], mdt, tag="v2a")
                nc.sync.dma_start(v2a[:, :, :, 64:65],
                                  ones_dram[:].bitcast(mdt))
                for j_t in range(NJT):
                    trp = tr_ps.tile([128, 128], mdt, tag="tr")
                    nc.tensor.transpose(
                        trp[:], qkvt[:, 2, j_t * 128:(j_t + 1) * 128],
                        ident_sb[:])
                    copy_alt(v2a[:, j_t, 0, 0:64], trp[:, 0:64])
                    copy_alt(v2a[:, j_t, 1, 0:64], trp[:, 64:128])

                # ---- attention per local head ----
                y_sb = yp.tile([128, T], mdt, tag="y")
                for h in range(HPC):
                    q_ap = qkvt[h * 64:(h + 1) * 64, 0, :]
                    k_ap = qkvt[h * 64:(h + 1) * 64, 1, :]
                    for i_sb in range(NSB):
                        yt = yo_ps.tile([65, SW], f32, tag="yo")
                        njt = 4 * (i_sb + 1)
                        for j_t in range(njt):
                            jtl = j_t - 4 * i_sb   # >=0 on the diagonal
                            diag = jtl >= 0
                            w = SW - jtl * 128 if diag else SW
                            i_lo = j_t * 128 if diag else i_sb * SW
                            sp = s_ps.tile([128, SW], f32, tag="s")
                            nc.tensor.matmul(
                                sp[:, :w],
                                lhsT=k_ap[:, j_t * 128:(j_t + 1) * 128],
                                rhs=q_ap[:, i_lo:i_lo + w],
                                start=True, stop=not diag,
                            )
                            if diag:
                                # causal mask added in-PSUM: += I.T @ trit
                                nc.tensor.matmul(
                                    sp[:, 0:128], lhsT=ident_sb[:],
                                    rhs=trit_sb[:], start=False, stop=True)
                            pt = ptp.tile([128, SW], mdt, tag="pt")
                            nc.scalar.activation(
                                pt[:, :w], sp[:, :w], Exp, scale=1.0 / D)
                            nc.tensor.matmul(
                                yt[:, SW - w:SW],
                                lhsT=v2a[:, j_t, h, :],
                                rhs=pt[:, :w],
                                start=(j_t == 0), stop=(j_t == njt - 1),
                            )
                        # normalize: y = yt[0:64] * bcast(1/yt[64])
                        # (denom row bounced to SBUF: the approx-recip
                        # custom op misreads PSUM/base-64 inputs on HW)
                        dnr = stats.tile([1, SW], f32, tag="dnr")
                        nc.vector.tensor_copy(dnr[:], yt[64:65, :])
                        rcp = stats.tile([1, SW], f32, tag="rcp")
                        nc.vector.reciprocal_approx_fast(out=rcp[:], in_=dnr[:])
                        rb = rbp.tile([64, SW], f32, tag="rb")
                        nc.gpsimd.partition_broadcast(rb[:], rcp[:])
                        nc.vector.tensor_mul(
                            y_sb[h * 64:(h + 1) * 64,
                                 i_sb * SW:(i_sb + 1) * SW],
                            yt[0:64, :], rb[:])

                # ---- proj partial ----
                for m_t in range(T // 128):
                    for c_h in range(C // SW):
                        op = yo_ps.tile([128, SW], f32, tag="yo")
                        nc.tensor.matmul(
                            op[:],
                            lhsT=y_sb[:, m_t * 128:(m_t + 1) * 128],
                            rhs=wp_sb[:, c_h * SW:(c_h + 1) * SW],
                            start=True, stop=True,
                        )
                        ost = stage.tile([128, SW], f32, tag="ost")
                        nc.vector.tensor_copy(ost[:], op[:])
                        nc.sync.dma_start(
                            out[b, m_t * 128:(m_t + 1) * 128,
                                c_h * SW:(c_h + 1) * SW], ost[:])

    nc.compile()
    return nc


def _get_nc():
    if "nc" not in _cache:
        _cache["nc"] = _build()
    return _cache["nc"]


def kernel(x, W_attn, b_attn, W_proj, b_proj):
    global LAST_RESULT
    from concourse.bass_utils import run_bass_kernel_spmd

    x = np.asarray(x, dtype=np.float32)
    W_attn = np.asarray(W_attn, dtype=np.float32)
    b_attn = np.asarray(b_attn, dtype=np.float32)
    W_proj = np.asarray(W_proj, dtype=np.float32)
    b_proj = np.asarray(b_proj, dtype=np.float32)

    nc = _get_nc()
    np_m = _np_mdt()

    xt = np.ascontiguousarray(x.transpose(0, 2, 1)).astype(np_m)
    in_maps = []
    for c in range(N_CORES):
        sl = slice(c * NL, (c + 1) * NL)
        w_shard = np.concatenate(
            [W_attn[sl], W_attn[C:2 * C][sl], W_attn[2 * C:][sl]], axis=0)
        # wqkv[p, k, n] = w_shard.T[k*128+p, n]
        wqkv = np.ascontiguousarray(
            w_shard.T.reshape(KT, 128, NT * 128).transpose(1, 0, 2)).astype(np_m)
        b_shard = np.concatenate(
            [b_attn[sl], b_attn[C:2 * C][sl], b_attn[2 * C:][sl]])
        bq = np.ascontiguousarray(b_shard.reshape(NT, 128).T)
        wp_c = np.ascontiguousarray(W_proj[:, sl].T).astype(np_m)
        in_maps.append({"xt": xt, "wqkv": wqkv, "bqkv": bq, "wp": wp_c})

    res = run_bass_kernel_spmd(nc, in_maps, core_ids=list(range(N_CORES)))
    LAST_RESULT = res

    acc = res.results[0]["out"].astype(np.float32)
    for c in range(1, N_CORES):
        acc = acc + res.results[c]["out"]
    return acc + b_proj


# revision 29
# speedup vs baseline: 1.0216x; 1.0160x over previous
"""Causal self-attention (B=4, T=2048, C=1024, H=16) on 8 TRN2 NeuronCores.

Sharding: tensor-parallel over heads. Each core owns 2 heads:
  - c_attn: output columns (q,k,v dims) for its heads  -> [384, 1024] shard
  - attention: embarrassingly parallel over (B, local heads)
  - c_proj: input rows for its heads -> partial [B,T,C] output, summed on host

Device layouts (host pre-transposed so every matmul contraction dim sits on
SBUF partitions):
  xt   [B, C, T]       x transposed; QKV matmul rhs tiles  [128 k, 512 tok]
  wqkv [128, 8, 384]   wqkv[p,k,n] = W_shard.T[k*128+p, n] (lhsT tiles)
  bqkv [128, 3]        per-partition bias, column n_t
  wp   [128, 1024]     wp[p,c] = W_proj[c, core*128+p]     (proj rhs)

Per-core pipeline per batch b:
  QKV^T [128, 3, 2048] = W.T @ x.T (+bias, ACT/DVE alternating)
  V2aug [128 tok, j_t, h, 65] = V^T transposed, ones column appended
  per head h, per 512-col i-superblock, per 128-row j tile (causal only):
    S^T = K_j^T.T @ Q^T        [128 j, w i] PSUM   (w shrinks on diagonal)
    P^T = exp(S^T/64 + mask)   ACT -> SBUF, directly the PV rhs
    Y^T[65, 512] += V2aug_j.T @ P^T    (row 64 = softmax denominator)
  y = Y^T[0:64] * bcast(approx 1/Y^T[64])   (DVE + gpsimd broadcast)
  proj partial = y_all.T @ Wp^T -> DMA out [B, T, C]
Host: out = sum(partials) + b_proj.
"""

import os
import sys

import numpy as np

os.environ.setdefault("MYCRO_LOCAL_CACHE", "1")
if "/opt/trn_rl_repo" not in sys.path:
    sys.path.insert(0, "/opt/trn_rl_repo")

B, T, C = 4, 2048, 1024
H, D = 16, 64
N_CORES = 8
HPC = H // N_CORES          # heads per core = 2
NL = HPC * D                # local width per q/k/v = 128
KT = C // 128               # 8 contraction tiles for QKV
NT = 3                      # q, k, v
SW = 512                    # i superblock width
NSB = T // SW               # 4 superblocks per batch
NJT = T // 128              # 16 j tiles per batch
NEG = -1.0e30

# matmul input dtype: bf16 (fastest), f32r (tf32-like), f32 (exact, 4x slow)
KDT = os.environ.get("KERNEL_DTYPE", "bf16")

_cache = {}
LAST_RESULT = None


def _np_mdt():
    if KDT == "bf16":
        import ml_dtypes
        return np.dtype(ml_dtypes.bfloat16)
    return np.dtype(np.float32)


def _build():
    import concourse.tile as tile
    from concourse import bacc, mybir

    dt = mybir.dt
    f32 = dt.float32
    mdt = {"bf16": dt.bfloat16, "f32r": dt.float32r, "f32": f32}[KDT]

    nc = bacc.Bacc("TRN2", target_bir_lowering=False, debug=False,
                   num_devices=N_CORES)

    xt = nc.dram_tensor("xt", [B, C, T], mdt, kind="ExternalInput").ap()
    wqkv = nc.dram_tensor("wqkv", [128, KT, NT * 128], mdt,
                          kind="ExternalInput").ap()
    bqkv = nc.dram_tensor("bqkv", [128, NT], f32, kind="ExternalInput").ap()
    wp = nc.dram_tensor("wp", [128, C], mdt, kind="ExternalInput").ap()
    out = nc.dram_tensor("out", [B, T, C], f32, kind="ExternalOutput").ap()

    np_m = _np_mdt() if KDT == "bf16" else np.float32
    ident_np = np.eye(128).astype(np_m)
    # S^T layout: rows x = j (keys), cols y = i (queries); keep j <= i
    trit_np = np.where(np.arange(128)[:, None] <= np.arange(128)[None, :],
                       np.float32(0.0), np.float32(NEG)).astype(np_m)
    ones_np = np.ones((128, NJT, HPC, 1)).astype(np_m)
    ident_dram = nc.inline_tensor(ident_np, name="ident").ap()
    trit_dram = nc.inline_tensor(trit_np, name="tritmask").ap()
    ones_dram = nc.inline_tensor(ones_np, name="onescol").ap()

    Exp = mybir.ActivationFunctionType.Exp

    _alt = [0]

    def copy_alt(dst, src):
        _alt[0] ^= 1
        if _alt[0]:
            nc.scalar.copy(dst, src)
        else:
            nc.vector.tensor_copy(dst, src)

    with tile.TileContext(nc) as tc:
        with (
            tc.tile_pool(name="consts", bufs=1) as consts,
            tc.tile_pool(name="xtp", bufs=1) as xtp,
            tc.tile_pool(name="qkvtp", bufs=2) as qkvtp,
            tc.tile_pool(name="yp", bufs=1) as yp,
            tc.tile_pool(name="v2p", bufs=2) as v2p,
            tc.tile_pool(name="ptp", bufs=10) as ptp,
            tc.tile_pool(name="stage", bufs=6) as stage,
            tc.tile_pool(name="stats", bufs=2) as stats,
            tc.tile_pool(name="rbp", bufs=2) as rbp,
            tc.tile_pool(name="qkv_ps", bufs=2, space="PSUM") as qkv_ps,
            tc.tile_pool(name="s_ps", bufs=3, space="PSUM") as s_ps,
            tc.tile_pool(name="tr_ps", bufs=1, space="PSUM") as tr_ps,
            tc.tile_pool(name="yo_ps", bufs=2, space="PSUM") as yo_ps,
        ):
            # HAM warm-up primer: dense dummy matmuls with no input deps so
            # the PE clock is at 2.4GHz by the time real work arrives.
            prime = consts.tile([128, SW], mdt if KDT != "f32r" else f32)
            nc.gpsimd.memset(prime[:], 0.25)
            for _ in range(0 if KDT == "f32r" else 24):
                pps = s_ps.tile([128, SW], f32, tag="s")
                nc.tensor.matmul(pps[:], lhsT=prime[:, 0:128], rhs=prime[:],
                                 start=True, stop=True)

            wqkv_sb = consts.tile([128, KT, NT * 128], mdt)
            nc.sync.dma_start(wqkv_sb[:], wqkv[:])
            wp_sb = consts.tile([128, C], mdt)
            nc.sync.dma_start(wp_sb[:], wp[:])
            bias_sb = consts.tile([128, NT], f32)
            nc.sync.dma_start(bias_sb[:], bqkv[:])
            ident_sb = consts.tile([128, 128], mdt)
            nc.sync.dma_start(ident_sb[:], ident_dram[:].bitcast(mdt))
            trit_sb = consts.tile([128, 128], mdt)
            nc.sync.dma_start(trit_sb[:], trit_dram[:].bitcast(mdt))
            for b in range(B):
                # ---- load x^T for this batch, [128, k, T] ----
                xt_sb = xtp.tile([128, KT, T], mdt, tag="xt")
                for k in range(KT):
                    nc.sync.dma_start(xt_sb[:, k, :],
                                      xt[b, k * 128:(k + 1) * 128, :])

                # ---- QKV^T = W.T @ x.T + bias ----
                qkvt = qkvtp.tile([128, NT, T], mdt, tag="qkvt")
                for n_t in range(NT):
                    for ts in range(T // SW):
                        ps = qkv_ps.tile([128, SW], f32, tag="qkv")
                        for k in range(KT):
                            nc.tensor.matmul(
                                ps[:],
                                lhsT=wqkv_sb[:, k, n_t * 128:(n_t + 1) * 128],
                                rhs=xt_sb[:, k, ts * SW:(ts + 1) * SW],
                                start=(k == 0), stop=(k == KT - 1),
                            )
                        _alt[0] ^= 1
                        if _alt[0]:
                            nc.scalar.activation(
                                qkvt[:, n_t, ts * SW:(ts + 1) * SW], ps[:],
                                mybir.ActivationFunctionType.Identity,
                                bias=bias_sb[:, n_t:n_t + 1], scale=1.0)
                        else:
                            nc.vector.tensor_scalar_add(
                                qkvt[:, n_t, ts * SW:(ts + 1) * SW], ps[:],
                                bias_sb[:, n_t:n_t + 1])

                # ---- V2aug: V^T transposed + ones column ----
                v2a = v2p.tile([128, NJT, HPC, 65], mdt, tag="v2a")
                nc.gpsimd.dma_start(v2a[:, :, :, 64:65],
                                    ones_dram[:].bitcast(mdt))
                for j_t in range(NJT):
                    trp = tr_ps.tile([128, 128], mdt, tag="tr")
                    nc.tensor.transpose(
                        trp[:], qkvt[:, 2, j_t * 128:(j_t + 1) * 128],
                        ident_sb[:])
                    copy_alt(v2a[:, j_t, 0, 0:64], trp[:, 0:64])
                    copy_alt(v2a[:, j_t, 1, 0:64], trp[:, 64:128])

                # ---- attention per local head ----
                y_sb = yp.tile([128, T], mdt, tag="y")
                for h in range(HPC):
                    q_ap = qkvt[h * 64:(h + 1) * 64, 0, :]
                    k_ap = qkvt[h * 64:(h + 1) * 64, 1, :]
                    for i_sb in range(NSB):
                        yt = yo_ps.tile([65, SW], f32, tag="yo")
                        njt = 4 * (i_sb + 1)
                        for j_t in range(njt):
                            jtl = j_t - 4 * i_sb   # >=0 on the diagonal
                            diag = jtl >= 0
                            w = SW - jtl * 128 if diag else SW
                            i_lo = j_t * 128 if diag else i_sb * SW
                            sp = s_ps.tile([128, SW], f32, tag="s")
                            nc.tensor.matmul(
                                sp[:, :w],
                                lhsT=k_ap[:, j_t * 128:(j_t + 1) * 128],
                                rhs=q_ap[:, i_lo:i_lo + w],
                                start=True, stop=not diag,
                            )
                            if diag:
                                # causal mask added in-PSUM: += I.T @ trit
                                nc.tensor.matmul(
                                    sp[:, 0:128], lhsT=ident_sb[:],
                                    rhs=trit_sb[:], start=False, stop=True)
                            pt = ptp.tile([128, SW], mdt, tag="pt")
                            nc.scalar.activation(
                                pt[:, :w], sp[:, :w], Exp, scale=1.0 / D)
                            nc.tensor.matmul(
                                yt[:, SW - w:SW],
                                lhsT=v2a[:, j_t, h, :],
                                rhs=pt[:, :w],
                                start=(j_t == 0), stop=(j_t == njt - 1),
                            )
                        # normalize: y = yt[0:64] * bcast(1/yt[64])
                        # (denom row bounced to SBUF: the approx-recip
                        # custom op misreads PSUM/base-64 inputs on HW)
                        dnr = stats.tile([1, SW], f32, tag="dnr")
                        nc.vector.tensor_copy(dnr[:], yt[64:65, :])
                        rcp = stats.tile([1, SW], f32, tag="rcp")
                        nc.vector.reciprocal_approx_fast(out=rcp[:], in_=dnr[:])
                        rb = rbp.tile([64, SW], f32, tag="rb")
                        nc.gpsimd.partition_broadcast(rb[:], rcp[:])
                        nc.vector.tensor_mul(
                            y_sb[h * 64:(h + 1) * 64,
                                 i_sb * SW:(i_sb + 1) * SW],
                            yt[0:64, :], rb[:])

                # ---- proj partial ----
                for m_t in range(T // 128):
                    for c_h in range(C // SW):
                        op = yo_ps.tile([128, SW], f32, tag="yo")
                        nc.tensor.matmul(
                            op[:],
                            lhsT=y_sb[:, m_t * 128:(m_t + 1) * 128],
                            rhs=wp_sb[:, c_h * SW:(c_h + 1) * SW],
                            start=True, stop=True,
                        )
                        ost = stage.tile([128, SW], f32, tag="ost")
                        nc.vector.tensor_copy(ost[:], op[:])
                        st_eng = nc.sync if b == B - 1 else nc.gpsimd
                        st_eng.dma_start(
                            out[b, m_t * 128:(m_t + 1) * 128,
                                c_h * SW:(c_h + 1) * SW], ost[:])

    nc.compile()
    return nc


def _get_nc():
    if "nc" not in _cache:
        _cache["nc"] = _build()
    return _cache["nc"]


def kernel(x, W_attn, b_attn, W_proj, b_proj):
    global LAST_RESULT
    from concourse.bass_utils import run_bass_kernel_spmd

    x = np.asarray(x, dtype=np.float32)
    W_attn = np.asarray(W_attn, dtype=np.float32)
    b_attn = np.asarray(b_attn, dtype=np.float32)
    W_proj = np.asarray(W_proj, dtype=np.float32)
    b_proj = np.asarray(b_proj, dtype=np.float32)

    nc = _get_nc()
    np_m = _np_mdt()

    xt = np.ascontiguousarray(x.transpose(0, 2, 1)).astype(np_m)
    in_maps = []
    for c in range(N_CORES):
        sl = slice(c * NL, (c + 1) * NL)
        w_shard = np.concatenate(
            [W_attn[sl], W_attn[C:2 * C][sl], W_attn[2 * C:][sl]], axis=0)
        # wqkv[p, k, n] = w_shard.T[k*128+p, n]
        wqkv = np.ascontiguousarray(
            w_shard.T.reshape(KT, 128, NT * 128).transpose(1, 0, 2)).astype(np_m)
        b_shard = np.concatenate(
            [b_attn[sl], b_attn[C:2 * C][sl], b_attn[2 * C:][sl]])
        bq = np.ascontiguousarray(b_shard.reshape(NT, 128).T)
        wp_c = np.ascontiguousarray(W_proj[:, sl].T).astype(np_m)
        in_maps.append({"xt": xt, "wqkv": wqkv, "bqkv": bq, "wp": wp_c})

    try:
        res = run_bass_kernel_spmd(nc, in_maps,
                                   core_ids=list(range(N_CORES)))
    except Exception:
        # one retry: transient NRT/device hiccups recover on re-run
        import time
        time.sleep(10)
        res = run_bass_kernel_spmd(nc, in_maps,
                                   core_ids=list(range(N_CORES)))
    LAST_RESULT = res

    acc = res.results[0]["out"].astype(np.float32)
    for c in range(1, N_CORES):
        acc = acc + res.results[c]["out"]
    return acc + b_proj


# revision 33
# speedup vs baseline: 1.0509x; 1.0288x over previous
"""Causal self-attention (B=4, T=2048, C=1024, H=16) on 8 TRN2 NeuronCores.

Sharding: tensor-parallel over heads. Each core owns 2 heads:
  - c_attn: output columns (q,k,v dims) for its heads  -> [384, 1024] shard
  - attention: embarrassingly parallel over (B, local heads)
  - c_proj: input rows for its heads -> partial [B,T,C] output, summed on host

Device layouts (host pre-transposed so every matmul contraction dim sits on
SBUF partitions):
  xt   [B, C, T]       x transposed; QKV matmul rhs tiles  [128 k, 512 tok]
  wqkv [128, 8, 384]   wqkv[p,k,n] = W_shard.T[k*128+p, n] (lhsT tiles)
  bqkv [128, 3]        per-partition bias, column n_t
  wp   [128, 1024]     wp[p,c] = W_proj[c, core*128+p]     (proj rhs)

Per-core pipeline per batch b:
  QKV^T [128, 3, 2048] = W.T @ x.T (+bias, ACT/DVE alternating)
  V2aug [128 tok, j_t, h, 65] = V^T transposed, ones column appended
  per head h, per 512-col i-superblock, per 128-row j tile (causal only):
    S^T = K_j^T.T @ Q^T        [128 j, w i] PSUM   (w shrinks on diagonal)
    P^T = exp(S^T/64 + mask)   ACT -> SBUF, directly the PV rhs
    Y^T[65, 512] += V2aug_j.T @ P^T    (row 64 = softmax denominator)
  y = Y^T[0:64] * bcast(approx 1/Y^T[64])   (DVE + gpsimd broadcast)
  proj partial = y_all.T @ Wp^T -> DMA out [B, T, C]
Host: out = sum(partials) + b_proj.
"""

import os
import sys

import numpy as np

os.environ.setdefault("MYCRO_LOCAL_CACHE", "1")
if "/opt/trn_rl_repo" not in sys.path:
    sys.path.insert(0, "/opt/trn_rl_repo")

B, T, C = 4, 2048, 1024
H, D = 16, 64
N_CORES = 8
HPC = H // N_CORES          # heads per core = 2
NL = HPC * D                # local width per q/k/v = 128
KT = C // 128               # 8 contraction tiles for QKV
NT = 3                      # q, k, v
SW = 512                    # i superblock width
NSB = T // SW               # 4 superblocks per batch
NJT = T // 128              # 16 j tiles per batch
NEG = -1.0e30

# matmul input dtype: bf16 (fastest), f32r (tf32-like), f32 (exact, 4x slow)
KDT = os.environ.get("KERNEL_DTYPE", "bf16")

_cache = {}
LAST_RESULT = None


def _np_mdt():
    if KDT == "bf16":
        import ml_dtypes
        return np.dtype(ml_dtypes.bfloat16)
    return np.dtype(np.float32)


def _build():
    import concourse.tile as tile
    from concourse import bacc, mybir

    dt = mybir.dt
    f32 = dt.float32
    mdt = {"bf16": dt.bfloat16, "f32r": dt.float32r, "f32": f32}[KDT]

    nc = bacc.Bacc("TRN2", target_bir_lowering=False, debug=False,
                   num_devices=N_CORES)

    xt = nc.dram_tensor("xt", [B, C, T], mdt, kind="ExternalInput").ap()
    wqkv = nc.dram_tensor("wqkv", [128, KT, NT * 128], mdt,
                          kind="ExternalInput").ap()
    bqkv = nc.dram_tensor("bqkv", [128, NT], f32, kind="ExternalInput").ap()
    wp = nc.dram_tensor("wp", [128, C], mdt, kind="ExternalInput").ap()
    out = nc.dram_tensor("out", [B, T, C], f32, kind="ExternalOutput").ap()

    np_m = _np_mdt() if KDT == "bf16" else np.float32
    ident_np = np.eye(128).astype(np_m)
    # S^T layout: rows x = j (keys), cols y = i (queries); keep j <= i
    trit_np = np.where(np.arange(128)[:, None] <= np.arange(128)[None, :],
                       np.float32(0.0), np.float32(NEG)).astype(np_m)
    ones_np = np.ones((128, NJT, HPC, 1)).astype(np_m)
    ident_dram = nc.inline_tensor(ident_np, name="ident").ap()
    trit_dram = nc.inline_tensor(trit_np, name="tritmask").ap()
    ones_dram = nc.inline_tensor(ones_np, name="onescol").ap()

    Exp = mybir.ActivationFunctionType.Exp

    _alt = [0]

    def copy_alt(dst, src):
        _alt[0] ^= 1
        if _alt[0]:
            nc.scalar.copy(dst, src)
        else:
            nc.vector.tensor_copy(dst, src)

    with tile.TileContext(nc) as tc:
        with (
            tc.tile_pool(name="consts", bufs=1) as consts,
            tc.tile_pool(name="xtp", bufs=2) as xtp,
            tc.tile_pool(name="qkvtp", bufs=3) as qkvtp,
            tc.tile_pool(name="yp", bufs=1) as yp,
            tc.tile_pool(name="v2p", bufs=2) as v2p,
            tc.tile_pool(name="ptp", bufs=10) as ptp,
            tc.tile_pool(name="stage", bufs=6) as stage,
            tc.tile_pool(name="stats", bufs=2) as stats,
            tc.tile_pool(name="rbp", bufs=2) as rbp,
            tc.tile_pool(name="qkv_ps", bufs=2, space="PSUM") as qkv_ps,
            tc.tile_pool(name="s_ps", bufs=3, space="PSUM") as s_ps,
            tc.tile_pool(name="tr_ps", bufs=1, space="PSUM") as tr_ps,
            tc.tile_pool(name="yo_ps", bufs=2, space="PSUM") as yo_ps,
        ):
            # HAM warm-up primer: dense dummy matmuls with no input deps so
            # the PE clock is at 2.4GHz by the time real work arrives.
            prime = consts.tile([128, SW], mdt if KDT != "f32r" else f32)
            nc.gpsimd.memset(prime[:], 0.25)
            for _ in range(0 if KDT == "f32r" else 24):
                pps = s_ps.tile([128, SW], f32, tag="s")
                nc.tensor.matmul(pps[:], lhsT=prime[:, 0:128], rhs=prime[:],
                                 start=True, stop=True)

            wqkv_sb = consts.tile([128, KT, NT * 128], mdt)
            nc.sync.dma_start(wqkv_sb[:], wqkv[:])
            wp_sb = consts.tile([128, C], mdt)
            nc.sync.dma_start(wp_sb[:], wp[:])
            bias_sb = consts.tile([128, NT], f32)
            nc.sync.dma_start(bias_sb[:], bqkv[:])
            ident_sb = consts.tile([128, 128], mdt)
            nc.sync.dma_start(ident_sb[:], ident_dram[:].bitcast(mdt))
            trit_sb = consts.tile([128, 128], mdt)
            nc.sync.dma_start(trit_sb[:], trit_dram[:].bitcast(mdt))
            for b in range(B):
                # ---- load x^T for this batch, [128, k, T] ----
                xt_sb = xtp.tile([128, KT, T], mdt, tag="xt")
                for k in range(KT):
                    nc.sync.dma_start(xt_sb[:, k, :],
                                      xt[b, k * 128:(k + 1) * 128, :])

                # ---- QKV^T = W.T @ x.T + bias ----
                qkvt = qkvtp.tile([128, NT, T], mdt, tag="qkvt")
                for n_t in range(NT):
                    for ts in range(T // SW):
                        ps = qkv_ps.tile([128, SW], f32, tag="qkv")
                        for k in range(KT):
                            nc.tensor.matmul(
                                ps[:],
                                lhsT=wqkv_sb[:, k, n_t * 128:(n_t + 1) * 128],
                                rhs=xt_sb[:, k, ts * SW:(ts + 1) * SW],
                                start=(k == 0), stop=(k == KT - 1),
                            )
                        _alt[0] ^= 1
                        if _alt[0]:
                            nc.scalar.activation(
                                qkvt[:, n_t, ts * SW:(ts + 1) * SW], ps[:],
                                mybir.ActivationFunctionType.Identity,
                                bias=bias_sb[:, n_t:n_t + 1], scale=1.0)
                        else:
                            nc.vector.tensor_scalar_add(
                                qkvt[:, n_t, ts * SW:(ts + 1) * SW], ps[:],
                                bias_sb[:, n_t:n_t + 1])

                # ---- V2aug: V^T transposed + ones column ----
                v2a = v2p.tile([128, NJT, HPC, 65], mdt, tag="v2a")
                nc.gpsimd.dma_start(v2a[:, :, :, 64:65],
                                    ones_dram[:].bitcast(mdt))
                for j_t in range(NJT):
                    trp = tr_ps.tile([128, 128], mdt, tag="tr")
                    nc.tensor.transpose(
                        trp[:], qkvt[:, 2, j_t * 128:(j_t + 1) * 128],
                        ident_sb[:])
                    copy_alt(v2a[:, j_t, 0, 0:64], trp[:, 0:64])
                    copy_alt(v2a[:, j_t, 1, 0:64], trp[:, 64:128])

                # ---- attention per local head ----
                y_sb = yp.tile([128, T], mdt, tag="y")
                for h in range(HPC):
                    q_ap = qkvt[h * 64:(h + 1) * 64, 0, :]
                    k_ap = qkvt[h * 64:(h + 1) * 64, 1, :]
                    for i_sb in range(NSB):
                        yt = yo_ps.tile([65, SW], f32, tag="yo")
                        njt = 4 * (i_sb + 1)
                        for j_t in range(njt):
                            jtl = j_t - 4 * i_sb   # >=0 on the diagonal
                            diag = jtl >= 0
                            w = SW - jtl * 128 if diag else SW
                            i_lo = j_t * 128 if diag else i_sb * SW
                            sp = s_ps.tile([128, SW], f32, tag="s")
                            nc.tensor.matmul(
                                sp[:, :w],
                                lhsT=k_ap[:, j_t * 128:(j_t + 1) * 128],
                                rhs=q_ap[:, i_lo:i_lo + w],
                                start=True, stop=not diag,
                            )
                            if diag:
                                # causal mask added in-PSUM: += I.T @ trit
                                nc.tensor.matmul(
                                    sp[:, 0:128], lhsT=ident_sb[:],
                                    rhs=trit_sb[:], start=False, stop=True)
                            pt = ptp.tile([128, SW], mdt, tag="pt")
                            nc.scalar.activation(
                                pt[:, :w], sp[:, :w], Exp, scale=1.0 / D)
                            nc.tensor.matmul(
                                yt[:, SW - w:SW],
                                lhsT=v2a[:, j_t, h, :],
                                rhs=pt[:, :w],
                                start=(j_t == 0), stop=(j_t == njt - 1),
                            )
                        # normalize: y = yt[0:64] * bcast(1/yt[64])
                        # (denom row bounced to SBUF: the approx-recip
                        # custom op misreads PSUM/base-64 inputs on HW)
                        dnr = stats.tile([1, SW], f32, tag="dnr")
                        nc.vector.tensor_copy(dnr[:], yt[64:65, :])
                        rcp = stats.tile([1, SW], f32, tag="rcp")
                        nc.vector.reciprocal_approx_fast(out=rcp[:], in_=dnr[:])
                        rb = rbp.tile([64, SW], f32, tag="rb")
                        nc.gpsimd.partition_broadcast(rb[:], rcp[:])
                        nc.vector.tensor_mul(
                            y_sb[h * 64:(h + 1) * 64,
                                 i_sb * SW:(i_sb + 1) * SW],
                            yt[0:64, :], rb[:])

                # ---- proj partial ----
                for m_t in range(T // 128):
                    for c_h in range(C // SW):
                        op = yo_ps.tile([128, SW], f32, tag="yo")
                        nc.tensor.matmul(
                            op[:],
                            lhsT=y_sb[:, m_t * 128:(m_t + 1) * 128],
                            rhs=wp_sb[:, c_h * SW:(c_h + 1) * SW],
                            start=True, stop=True,
                        )
                        ost = stage.tile([128, SW], f32, tag="ost")
                        nc.vector.tensor_copy(ost[:], op[:])
                        st_eng = nc.sync if b == B - 1 else nc.gpsimd
                        st_eng.dma_start(
                            out[b, m_t * 128:(m_t + 1) * 128,
                                c_h * SW:(c_h + 1) * SW], ost[:])

    nc.compile()
    return nc


def _get_nc():
    if "nc" not in _cache:
        _cache["nc"] = _build()
    return _cache["nc"]


def kernel(x, W_attn, b_attn, W_proj, b_proj):
    global LAST_RESULT
    from concourse.bass_utils import run_bass_kernel_spmd

    x = np.asarray(x, dtype=np.float32)
    W_attn = np.asarray(W_attn, dtype=np.float32)
    b_attn = np.asarray(b_attn, dtype=np.float32)
    W_proj = np.asarray(W_proj, dtype=np.float32)
    b_proj = np.asarray(b_proj, dtype=np.float32)

    nc = _get_nc()
    np_m = _np_mdt()

    xt = np.ascontiguousarray(x.transpose(0, 2, 1)).astype(np_m)
    in_maps = []
    for c in range(N_CORES):
        sl = slice(c * NL, (c + 1) * NL)
        w_shard = np.concatenate(
            [W_attn[sl], W_attn[C:2 * C][sl], W_attn[2 * C:][sl]], axis=0)
        # wqkv[p, k, n] = w_shard.T[k*128+p, n]
        wqkv = np.ascontiguousarray(
            w_shard.T.reshape(KT, 128, NT * 128).transpose(1, 0, 2)).astype(np_m)
        b_shard = np.concatenate(
            [b_attn[sl], b_attn[C:2 * C][sl], b_attn[2 * C:][sl]])
        bq = np.ascontiguousarray(b_shard.reshape(NT, 128).T)
        wp_c = np.ascontiguousarray(W_proj[:, sl].T).astype(np_m)
        in_maps.append({"xt": xt, "wqkv": wqkv, "bqkv": bq, "wp": wp_c})

    try:
        res = run_bass_kernel_spmd(nc, in_maps,
                                   core_ids=list(range(N_CORES)))
    except Exception:
        # one retry: transient NRT/device hiccups recover on re-run
        import time
        time.sleep(10)
        res = run_bass_kernel_spmd(nc, in_maps,
                                   core_ids=list(range(N_CORES)))
    LAST_RESULT = res

    acc = res.results[0]["out"].astype(np.float32)
    for c in range(1, N_CORES):
        acc = acc + res.results[c]["out"]
    return acc + b_proj
